# revision 4
# baseline (speedup 1.0000x reference)
"""Trainium2 Bass kernel for nn_MemristorArray (B=128, I=512, O=512).

Math (see reference):
  low = poly(poly_low, x); high = poly(poly_high, x); d = high - low
  g2[b,i] = 4*KBT*BW/(|x|+eps) + 2*e*BW
  out[b,o] = sum_i low[b,i] + (d @ r)[b,o]
           + sum_i noise[i,o] * sqrt(g2[b,i] * |low[b,i] + d[b,i]*r[i,o]|)

Sharding: data-parallel over batch, 16 rows per core on 8 cores. Host computes
the tiny per-(b,i) tables (poly eval, g2 folding); the O(B*I*O) work runs on
device. Per core, the [16,512,512] virtual tensor is 64 tiles of
[128 i-partitions x 512 o], grouped 4-per-batch-row into [128,2048] quads:

  affine+abs  a = |r*(g2*d) + (g2*low)|   -> custom DVE op (latched per-
              partition scalars; the stock TensorScalarPtr re-reads its
              scalar per element and runs 2.3ns/elem) for 3 of 4 chunks,
              ACT Abs-activation (scale/bias) for the 4th (load balance)
  sqrt        u = Sqrt(a)                 -> ACT, one op per [128,2048] quad
  noise mul   w = u * noise               -> DVE TT bf16 per quad; a subset of
              batch rows goes to GPSIMD instead (load balance)
  reduce_i    acc[b,:] += onehot(b).T @ w -> PE, one matmul per tile into a
              single [16,512] PSUM tile (one-hot column sliced from a shifted
              pattern), accumulated with the 4 f32 main-term d @ r matmuls
  sum_i low   enters as the per-partition bias of the final PSUM->SBUF copy.
"""
import numpy as np
import ml_dtypes
from contextlib import ExitStack

import concourse.bass as bass
import concourse.tile as tile
import concourse.dve_ops as dve_ops
from concourse import bacc, mybir
from concourse.bass_utils import run_bass_kernel_spmd
from concourse.dve_spec import Spec, Src0, C0, C1, Zero, maxx, lower, _has_src1
from concourse.dve_uop import DveOpSpec

B, I, O = 128, 512, 512
NCORES = 8
BPC = B // NCORES        # 16 batch rows per core
CH = I // 128            # 4 i-chunks of 128 partitions
f32 = mybir.dt.float32
bf16 = mybir.dt.bfloat16

BW = 1e-08
KBT = 1.380649e-23 * 300.0
EPS = 1e-12
C1_J = 4.0 * KBT * BW
C2_S = 2.0 * float(np.e) * BW

# Load-balance knobs: chunk whose affine runs on ACT instead of DVE, and
# batch rows whose noise-multiply runs on GPSIMD instead of DVE.
ACT_AFFINE_CHUNKS = (0,)
POOL_TT_ROWS = frozenset((0, 3, 6, 9, 12, 15))

PROFILE = False
TRACE_KW = {}
LAST_RESULTS = None

_BUILT = None
_NOISE = None


def _register_affine_abs():
    name = "MEMR_AFFINE_ABS"
    for op in dve_ops.OPS:
        if op.name == name:
            return op
    t = Src0 * C0 + C1
    spec = Spec(body=maxx(t, Zero - t),
                reference=lambda in0, in1, c0, c1, c2: np.abs(in0 * c0 + c1))
    row = dve_ops._CUSTOM_DVE_ROW_BASE + len(dve_ops.OPS)
    assert row < 0x20
    dve_ops._SUB_OPCODE_FOR_NAME[name] = row
    shas = {}
    for ver in ("v3", "v4"):
        u = lower(spec, ver=ver)
        shas[ver] = DveOpSpec(name=name, opcode=row, uops=u,
                              rd1_en=_has_src1(spec)).sha(ver)
    op = dve_ops.DveOp(name, spec, False, uops_sha=shas)
    dve_ops.OPS.append(op)
    dve_ops.CUSTOM_DVE_SPECS[name] = spec
    return op


AFFINE_ABS = _register_affine_abs()


def _build():
    nc = bacc.Bacc("TRN2", target_bir_lowering=False, debug=False)
    r32_d = nc.dram_tensor("r32", [I, O], f32, kind="ExternalInput")
    nz_d = nc.dram_tensor("nz", [I, O], bf16, kind="ExternalInput")
    sc_d = nc.dram_tensor("sc", [128, CH * BPC], f32, kind="ExternalInput")
    bi_d = nc.dram_tensor("bi", [128, CH * BPC], f32, kind="ExternalInput")
    dt_d = nc.dram_tensor("dt", [128, CH * BPC], f32, kind="ExternalInput")
    z_d = nc.dram_tensor("z", [128, 2 * BPC - 1], bf16, kind="ExternalInput")
    sl_d = nc.dram_tensor("sl", [BPC, 1], f32, kind="ExternalInput")
    out_d = nc.dram_tensor("out", [BPC, O], f32, kind="ExternalOutput")

    with tile.TileContext(nc) as tc, ExitStack() as ctx:
        singles = ctx.enter_context(tc.tile_pool(name="singles", bufs=1))
        apool = ctx.enter_context(tc.tile_pool(name="a", bufs=3))
        upool = ctx.enter_context(tc.tile_pool(name="u", bufs=3))
        wpool = ctx.enter_context(tc.tile_pool(name="w", bufs=3))
        pp = ctx.enter_context(tc.tile_pool(name="ps", bufs=1, space="PSUM"))

        r32 = singles.tile([128, CH * O], f32)
        nz = singles.tile([128, CH * O], bf16)
        for c in range(CH):
            nc.sync.dma_start(out=r32[:, c * O:(c + 1) * O],
                              in_=r32_d.ap()[c * 128:(c + 1) * 128, :])
            nc.sync.dma_start(out=nz[:, c * O:(c + 1) * O],
                              in_=nz_d.ap()[c * 128:(c + 1) * 128, :])
        sc = singles.tile([128, CH * BPC], f32)
        nc.sync.dma_start(out=sc, in_=sc_d.ap())
        bi = singles.tile([128, CH * BPC], f32)
        nc.sync.dma_start(out=bi, in_=bi_d.ap())
        dt = singles.tile([128, CH * BPC], f32)
        nc.sync.dma_start(out=dt, in_=dt_d.ap())
        z = singles.tile([128, 2 * BPC - 1], bf16)
        nc.sync.dma_start(out=z, in_=z_d.ap())
        sl = singles.tile([BPC, 1], f32)
        nc.sync.dma_start(out=sl, in_=sl_d.ap())

        acc = pp.tile([BPC, O], f32)
        for c in range(CH):
            nc.tensor.matmul(acc, dt[:, c * BPC:(c + 1) * BPC],
                             r32[:, c * O:(c + 1) * O],
                             start=(c == 0), stop=False)

        n_mm = 0
        for m in range(BPC):
            a = apool.tile([128, CH * O], bf16)
            for c in range(CH):
                col = c * BPC + m
                if c in ACT_AFFINE_CHUNKS:
                    nc.scalar.activation(
                        out=a[:, c * O:(c + 1) * O],
                        in_=r32[:, c * O:(c + 1) * O],
                        func=mybir.ActivationFunctionType.Abs,
                        bias=bi[:, col:col + 1], scale=sc[:, col:col + 1])
                else:
                    nc.vector._custom_dve(
                        AFFINE_ABS,
                        out=a[:, c * O:(c + 1) * O],
                        in0=r32[:, c * O:(c + 1) * O],
                        s0=sc[:, col:col + 1], s1=bi[:, col:col + 1])
            u = upool.tile([128, CH * O], bf16)
            nc.scalar.activation(out=u, in_=a,
                                 func=mybir.ActivationFunctionType.Sqrt)
            w = wpool.tile([128, CH * O], bf16)
            if m in POOL_TT_ROWS:
                nc.gpsimd.tensor_tensor(w, u, nz, mybir.AluOpType.mult)
            else:
                nc.vector.tensor_mul(w, u, nz)
            for c in range(CH):
                n_mm += 1
                nc.tensor.matmul(acc, z[:, BPC - 1 - m:2 * BPC - 1 - m],
                                 w[:, c * O:(c + 1) * O],
                                 start=False, stop=(n_mm == BPC * CH))

        outsb = singles.tile([BPC, O], f32)
        nc.scalar.activation(out=outsb, in_=acc,
                             func=mybir.ActivationFunctionType.Identity,
                             bias=sl, scale=1.0)
        nc.sync.dma_start(out=out_d.ap(), in_=outsb)

    nc.compile()
    return nc


def _get_noise():
    # Reproduce the reference's fixed noise draw on the same default backend
    # the reference would use; fall back to CPU if that fails.
    import jax
    import jax.numpy as jnp
    try:
        n = np.asarray(jax.random.normal(jax.random.key(42), (I, O),
                                         dtype=jnp.float32))
    except Exception:
        f = jax.jit(lambda: jax.random.normal(jax.random.key(42), (I, O),
                                              dtype=jnp.float32), backend="cpu")
        n = np.asarray(f())
    return n


def kernel(inputs, poly_low, poly_high, r):
    global _BUILT, _NOISE, LAST_RESULTS
    if _BUILT is None:
        _BUILT = _build()
    if _NOISE is None:
        _NOISE = _get_noise()

    x = inputs.astype(np.float64)
    pl = poly_low.astype(np.float64)
    ph = poly_high.astype(np.float64)
    low = np.polynomial.polynomial.polyval(x, pl)
    high = np.polynomial.polynomial.polyval(x, ph)
    d = high - low
    g2 = C1_J / (np.abs(x) + EPS) + C2_S

    sc_full = (g2 * d).astype(np.float32)         # [B, I]
    bi_full = (g2 * low).astype(np.float32)       # [B, I]
    dt_full = d.astype(np.float32)                # [B, I]
    sl_full = low.sum(axis=1).astype(np.float32)  # [B]

    r32 = np.ascontiguousarray(r.astype(np.float32))
    nzb = _NOISE.astype(ml_dtypes.bfloat16)
    z = np.zeros((128, 2 * BPC - 1), dtype=ml_dtypes.bfloat16)
    z[:, BPC - 1] = 1.0

    def pack(full, k):
        sub = full[k * BPC:(k + 1) * BPC, :]              # [BPC, I]
        return np.ascontiguousarray(
            sub.T.reshape(CH, 128, BPC).transpose(1, 0, 2).reshape(128, CH * BPC))

    in_maps = []
    for k in range(NCORES):
        in_maps.append(dict(
            r32=r32, nz=nzb, z=z,
            sc=pack(sc_full, k), bi=pack(bi_full, k), dt=pack(dt_full, k),
            sl=np.ascontiguousarray(sl_full[k * BPC:(k + 1) * BPC, None]),
        ))

    res = run_bass_kernel_spmd(_BUILT, in_maps, core_ids=list(range(NCORES)),
                               trace=PROFILE, **TRACE_KW)
    LAST_RESULTS = res
    out = np.concatenate([res.results[k]["out"] for k in range(NCORES)], axis=0)
    return np.ascontiguousarray(out.astype(np.float32))
